# revision 74
# baseline (speedup 1.0000x reference)
"""Trainium2 Bass kernel for nn_Decoder (GRU decoder with clipped-delta
position integration).

Strategy
--------
Data-parallel over the batch N=16384: 8 cores x 2048 rows, feature-major
layout (W_hh @ h streams h as the moving operand, weights stationary):

  h      [HID=256, 2048]  as SBUF [128, 2, 512] x4 chunks  (K-tile, batch)
  gates  [768, chunk=512] in PSUM, gate-major
  x_c    [8, 512] per chunk: rows 0-1 = prev delta, 2-6 = ctx, 7 = ONE
         (the ones-row carries the rz bias through the x-tail matmul).

The T loop runs as a tc.For_i HARDWARE loop (unroll=4 steps per
iteration) so the NEFF size is O(1) in T: per-call NEFF reload, not HW
compute, dominated the fully-unrolled build (~23 ms/step measured vs
~0.5-2 ms/step with the loop). Per-step cost in this axon environment is
dominated by cross-engine semaphore waits, so the structure minimizes
instructions and engine crossings:

  P1: 4 rz M-tiles accumulate into ONE 4-bank PSUM tile; the K=8 x-tail
      adds W_ih@x plus the bias via the ones-row; a single [128,4,512]
      sigmoid evicts all four (bias pre-added, so no per-tile ACT bias).
  P2/P3: 2-bank pair tiles; npre = (P2+b_ihn) + r*(P3+b_hhn) via two STT
      on DVE; one [128,2,512] tanh.
  h-update: h = n + z*(h-n) entirely on DVE (in-engine deps are free);
      fp32 master + bf16 shadow written by two DVE adds. GPSIMD does no
      compute at all - it is left free for SWDGE DMA descriptor work.
  delta = W_out @ h_new -> spread [128,512] tile (pairs at partitions
      32c); pair-sum matmul + quake-seed + 2 Newton rsqrt steps give the
      clip scale; clipped delta feeds back into x_c (bf16) and is written
      as f16 into a [128, unroll, 512] stage tile (32-aligned DVE writes).
  Output: CLIPPED DELTAS, 4 DMAs per loop iteration (not per step) from
      the stage tile; the host integrates positions with an f64 cumsum.
      PSUM: p14(4) + p2p(2) + p3p(2) = 8 banks; pd/pu recycle p14.

Matmul operands are bf16 (fp8 DoubleRow path exists behind K_FP8=1 but
costs ~1.6e-2 rel err at T=8 - too close to the 2e-2 gate). Weights and
the norm matmul stay as in the baseline. absmax error ~4.6e-3 relative
to absmax(reference) at T=96.
"""

import sys

for _p in ("/opt/trn_rl_repo",):
    if _p not in sys.path:
        sys.path.insert(0, _p)

import numpy as np

import concourse.bass as bass
import concourse.tile as tile
from concourse.bacc import Bacc
from concourse import mybir
from concourse.bass_utils import run_bass_kernel_spmd

F32 = mybir.dt.float32
F32R = mybir.dt.float32r
BF16 = mybir.dt.bfloat16
FP8 = mybir.dt.float8e4
I32 = mybir.dt.int32
AF = mybir.ActivationFunctionType
OP = mybir.AluOpType
PM = mybir.MatmulPerfMode

# fp8e4m3 path: W_hh/W_out/W_ih-tails scaled by WSCALE on host so U(-1/16,1/16)
# weights land in fp8 normal range; compensated by 1/WSCALE at PSUM eviction
# (free on the ACT scale operand; bv n-gate bias columns pre-scaled instead).
import os as _os
USE_FP8 = _os.environ.get("K_FP8", "0") == "1"
WSCALE = 64.0

HID = 256
CTX_DIM = 5
V_MAX = 10.1415
DT = 0.093
MS = V_MAX * DT  # max_step
N_CORES = 8
MAGIC = 0x5F3759DF - 0x400000  # quake magic adjusted for input u' = -0.5*u


QSCALE = 126.0 / MS    # uint8 delta quantization: q = round(d*QSCALE)+128
QOFF = 128.0           # HW f32->uint8 cast rounds half-even (probed); the
                       # CoreSim cast truncates, so sim u8 output drifts
                       # -0.5 LSB/step - validate the u8 path on HW only
Q4S = 6.97 / MS        # 4-bit scale: |d*Q4S + e| <= 7.47 -> round stays in
                       # [-7, 7]; error feedback bounds pos err to 0.5 LSB
RMAGIC = 12582912.0    # 1.5*2^23: a+RMAGIC-RMAGIC = round-half-even(a)


def build_module(T: int, nloc: int, unroll: int = 0, no_out: bool = False,
                 static_out: bool = False, out_u8: bool = False,
                 out_u4: bool = False):
    """Trace the Bass/Tile module for one core (nloc batch columns)."""
    CH = nloc // 512  # column chunks of 512
    assert nloc % 512 == 0
    if out_u4:
        assert unroll > 0 and unroll % 2 == 0
        out_u8 = False

    F16 = mybir.dt.float16
    HDT = FP8 if USE_FP8 else BF16

    nc = Bacc()

    # ---- DRAM I/O ----
    h0_d = nc.dram_tensor("h0", [2, 128, nloc], F32, kind="ExternalInput")
    h0b_d = nc.dram_tensor("h0b", [2, 128, nloc], HDT, kind="ExternalInput")
    x0_d = nc.dram_tensor("x0i", [8, nloc], BF16, kind="ExternalInput")
    pos0_d = nc.dram_tensor("pos0", [128, 512], F32, kind="ExternalInput")
    wh_d = nc.dram_tensor("wh", [2, 128, 768], HDT, kind="ExternalInput")
    wt_d = nc.dram_tensor("wt", [8, 8, 128], BF16, kind="ExternalInput")
    wo_d = nc.dram_tensor("wo", [2, 128, 2], HDT, kind="ExternalInput")
    wd2_d = nc.dram_tensor("wd2", [128, 128], F32, kind="ExternalInput")
    bv_d = nc.dram_tensor("bv", [128, 8], F32, kind="ExternalInput")
    bpk_d = nc.dram_tensor("bpk", [2, 1], F32, kind="ExternalInput")
    # unroll>0: batched output [n_iter, 8, unroll, 512] (one DMA per
    # iteration); unroll<=0: per-step [T, 2CH, 512].
    ODT = mybir.dt.uint8 if (out_u8 or out_u4) else F16
    ucols = (unroll // 2) if out_u4 else unroll
    if unroll > 0:
        n_it = 1 if no_out else T // unroll
        out_d = nc.dram_tensor("out", [n_it, 2 * CH, ucols, 512], ODT,
                               kind="ExternalOutput")
    else:
        out_T = 1 if no_out else T
        out_d = nc.dram_tensor("out", [out_T, 2 * CH, 512], F16,
                               kind="ExternalOutput")

    with tile.TileContext(nc) as tc:
        import contextlib

        ctx = contextlib.ExitStack()
        with ctx:
            singles = ctx.enter_context(tc.tile_pool(name="singles", bufs=1))
            h_c = []
            x_c = []
            hb_c = []
            for c in range(CH):
                h_c.append(singles.tile([128, 2, 512], F32, tag=f"h{c}", name=f"h{c}"))
                x_c.append(singles.tile([8, 512], BF16, tag=f"x{c}", name=f"x{c}"))
                hb_c.append(singles.tile([128, 2, 512], HDT, tag=f"hb{c}", name=f"hb{c}"))
            pos = singles.tile([128, 512], F32, tag="pos", name="pos") \
                if unroll <= 0 else None
            dbtw = singles.tile([128, 512], F32, tag="dbtw", name="dbtw")
            eq = qprev = None
            if (out_u8 or out_u4) and unroll > 0:
                # quantization residual (error feedback): bounds the
                # host-side cumsum error to 0.5 LSB regardless of T
                eq = singles.tile([128, 512], F32, tag="eq", name="eq")
                nc.vector.memset(eq, 0.0)
            if out_u4:
                # even-step rounded q values await the odd-step nibble pack
                qprev = singles.tile([128, 512], F32, tag="qprev",
                                     name="qprev")
            wh = singles.tile([128, 2, 768], HDT, tag="wh", name="wh")
            wt = singles.tile([8, 8, 128], BF16, tag="wt", name="wt")
            wo = singles.tile([128, 2, 2], HDT, tag="wo", name="wo")
            wd2 = singles.tile([128, 128], F32, tag="wd2", name="wd2")
            bv = singles.tile([128, 8], F32, tag="bv", name="bv")
            bpk = singles.tile([2, 1], F32, tag="bpk", name="bpk")
            nc.vector.memset(dbtw, 0.0)

            # initial loads
            for c in range(CH):
                cs = slice(c * 512, (c + 1) * 512)
                nc.sync.dma_start(
                    out=h_c[c],
                    in_=h0_d[:, :, :].transpose([1, 0, 2])[:, :, cs])
                nc.sync.dma_start(
                    out=hb_c[c],
                    in_=h0b_d[:, :, :].transpose([1, 0, 2])[:, :, cs])
                nc.sync.dma_start(out=x_c[c], in_=x0_d[:, :][:, cs])
            if pos is not None:
                nc.sync.dma_start(out=pos, in_=pos0_d[:, :])
            nc.sync.dma_start(out=wh, in_=wh_d[:, :, :].transpose([1, 0, 2]))
            nc.sync.dma_start(out=wt, in_=wt_d[:, :, :])
            nc.sync.dma_start(out=wo, in_=wo_d[:, :, :].transpose([1, 0, 2]))
            nc.sync.dma_start(out=wd2, in_=wd2_d[:, :])
            nc.sync.dma_start(out=bv, in_=bv_d[:, :])
            nc.sync.dma_start(out=bpk, in_=bpk_d[:, :])

            # PSUM pools: p14 (4 banks, also recycled for pd/pu), p2 pair,
            # p3 pair (2 banks each) = 8 banks exactly
            pp1 = ctx.enter_context(tc.tile_pool(name="pp1", bufs=1, space="PSUM"))
            ppc = ctx.enter_context(tc.tile_pool(name="ppc", bufs=1, space="PSUM"))
            ppe = ctx.enter_context(tc.tile_pool(name="ppe", bufs=1, space="PSUM"))
            sb = ctx.enter_context(tc.tile_pool(name="sb", bufs=3))
            sbs = ctx.enter_context(tc.tile_pool(name="sbs", bufs=3))

            def step(t_idx, stage=None, stage_j=0):
                for c in range(CH):
                    hc = h_c[c]
                    hb = hb_c[c]
                    xc = x_c[c]
                    # --- P1: rz preactivations, 4 M-tiles in ONE 4-bank PSUM
                    # tile; rz bias rides the ones-row (x row 7) through the
                    # x-tail matmul so a SINGLE sigmoid evicts all 4 tiles ---
                    inv_s = (1.0 / WSCALE) if USE_FP8 else 1.0
                    rzs = sb.tile([128, 4, 512], F32, tag="rzs", name="rzs")
                    p14 = pp1.tile([128, 4, 512], F32, tag="p14", name="p14")
                    for mt in range(4):
                        ms_ = slice(mt * 128, (mt + 1) * 128)
                        if USE_FP8:
                            nc.tensor.matmul(
                                p14[:, mt, :], wh[:, :, ms_], hb[:, :, :],
                                start=True, stop=False, perf_mode=PM.DoubleRow)
                        else:
                            nc.tensor.matmul(
                                p14[:, mt, :], wh[:, 0, ms_],
                                hb[:, 0, :], start=True, stop=False)
                            nc.tensor.matmul(
                                p14[:, mt, :], wh[:, 1, ms_],
                                hb[:, 1, :], start=False, stop=False)
                        nc.tensor.matmul(
                            p14[:, mt, :], wt[0:8, mt, :],
                            xc[0:8, :],
                            start=False, stop=True)
                    nc.scalar.activation(rzs, p14, AF.Sigmoid, scale=inv_s)
                    # --- P2: i_n, P3: h_n as 2-bank pair tiles (fp8: both
                    # carry a WSCALE factor; n-gate bias columns pre-scaled,
                    # tanh eviction divides back out via the ACT scale) ---
                    p2p = ppc.tile([128, 2, 512], F32, tag="p2p", name="p2p")
                    for i in range(2):
                        nc.tensor.matmul(
                            p2p[:, i, :], wt[0:8, 4 + i, :],
                            xc[0:8, :],
                            start=True, stop=True)
                    p3p = ppe.tile([128, 2, 512], F32, tag="p3p", name="p3p")
                    for i in range(2):
                        ms_ = slice(512 + i * 128, 512 + (i + 1) * 128)
                        if USE_FP8:
                            nc.tensor.matmul(
                                p3p[:, i, :], wh[:, :, ms_], hb[:, :, :],
                                start=True, stop=False, perf_mode=PM.DoubleRow)
                        else:
                            nc.tensor.matmul(
                                p3p[:, i, :], wh[:, 0, ms_],
                                hb[:, 0, :], start=True, stop=False)
                            nc.tensor.matmul(
                                p3p[:, i, :], wh[:, 1, ms_],
                                hb[:, 1, :], start=False, stop=False)
                        # b_hhn rides the ones-row through a bias-only tail
                        nc.tensor.matmul(
                            p3p[:, i, :], wt[0:8, 6 + i, :],
                            xc[0:8, :],
                            start=False, stop=True)
                    # --- npre = (P2+b_ihn) + r*(P3+b_hhn), biases already in
                    # PSUM: two full [128,2,512] DVE ops; n = tanh ---
                    npre = sb.tile([128, 2, 512], F32, tag="npre", name="npre")
                    t1 = sbs.tile([128, 2, 512], F32, tag="t1", name="t1")
                    nc.vector.tensor_mul(t1, p3p, rzs[:, 0:2, :])
                    nc.vector.tensor_add(npre, p2p, t1)
                    n_t = sb.tile([128, 2, 512], F32, tag="n", name="n")
                    nc.scalar.activation(n_t, npre, AF.Tanh, scale=inv_s)
                    # --- h = n + z*(h-n): whole chain on DVE as full
                    # [128,2,512] ops (both K-tiles per instruction; the z
                    # slots 2:4 of rzs line up with the kt axis). In-engine
                    # deps are free; cross-engine semaphore waits are the
                    # dominant per-step cost here, so gpsimd does no compute
                    # at all (it still runs SWDGE for DMAs).
                    t_d = sbs.tile([128, 2, 512], F32, tag="t2", name="t2")
                    nc.vector.tensor_sub(t_d, hc, n_t)
                    u_t = sbs.tile([128, 2, 512], F32, tag="u2", name="u2")
                    nc.vector.tensor_mul(u_t, rzs[:, 2:4, :], t_d)
                    nc.vector.tensor_add(hc, n_t, u_t)
                    nc.vector.tensor_add(hb, n_t, u_t)
                    # --- delta = W_out @ h_new, spread eviction. Reuses the
                    # p3p banks (consumed by the t1 mul, which is earlier
                    # than the sigmoid) so the wo->evict chain does NOT gate
                    # the next chunk's P1 matmuls through the p14 pool ---
                    pdb = ppe.tile([128, 2, 512], F32, tag="p3p", name="pdb")
                    pd = pdb[0:2, 0, :]
                    if USE_FP8:
                        nc.tensor.matmul(pd, wo[:, :, :], hb[:, :, :],
                                         start=True, stop=True,
                                         perf_mode=PM.DoubleRow)
                        nc.vector.tensor_scalar(
                            dbtw[32 * c:32 * c + 2, :], pd, inv_s, bpk[0:2, :],
                            op0=OP.mult, op1=OP.add)
                    else:
                        nc.tensor.matmul(pd, wo[:, 0, :],
                                         hb[:, 0, :],
                                         start=True, stop=False)
                        nc.tensor.matmul(pd, wo[:, 1, :],
                                         hb[:, 1, :],
                                         start=False, stop=True)
                        nc.vector.tensor_scalar(
                            dbtw[32 * c:32 * c + 2, :], pd, bpk[0:2, :], None,
                            op0=OP.add)

                # ---- clip: s = min(MS/||delta||, 1), spread [128, 512] ----
                sqv = sbs.tile([128, 512], F32, tag="sqv", name="sqv")
                nc.vector.tensor_mul(sqv, dbtw, dbtw)
                pub = ppe.tile([128, 2, 512], F32, tag="p3p", name="pub")
                pu = pub[:, 0, :]
                nc.tensor.matmul(pu, wd2, sqv, start=True, stop=True)
                s1i = sbs.tile([128, 512], I32, tag="s1i", name="s1i")
                nc.vector.tensor_scalar(
                    s1i, pu.bitcast(I32), 1, 0x3FFFFFFF,
                    op0=OP.logical_shift_right, op1=OP.bitwise_and)
                y0i = sbs.tile([128, 512], I32, tag="y0i", name="y0i")
                nc.vector.tensor_scalar(
                    y0i, s1i, MAGIC, -1, op0=OP.subtract, op1=OP.mult)
                y = y0i.bitcast(F32)
                ys = []
                for it in range(2):
                    m_t = sbs.tile([128, 512], F32, tag=f"m{it}", name=f"m{it}")
                    nc.vector.tensor_mul(m_t, y, y)
                    m2_t = sbs.tile([128, 512], F32, tag=f"m2{it}", name=f"m2{it}")
                    nc.vector.tensor_mul(m2_t, m_t, pu)
                    y2_t = sbs.tile([128, 512], F32, tag=f"y2{it}", name=f"y2{it}")
                    nc.vector.scalar_tensor_tensor(
                        y2_t, m2_t, 1.5, y, op0=OP.add, op1=OP.mult)
                    y = y2_t
                    ys.append(y)
                    if it == 0:
                        # x feedback tolerates 1-Newton precision (it is
                        # bf16-rounded anyway) -> unblock next step early
                        smin1 = sbs.tile([128, 512], F32, tag="smin1",
                                         name="smin1")
                        nc.vector.tensor_scalar(
                            smin1, y, 1.0, None, op0=OP.min)
                        for c in range(CH):
                            nc.vector.tensor_mul(
                                x_c[c][0:2, :], smin1[32 * c:32 * c + 2, :],
                                dbtw[32 * c:32 * c + 2, :])
                # output keeps the 2-Newton value
                smin = sbs.tile([128, 512], F32, tag="smin", name="smin")
                if out_u8 or out_u4:
                    nc.vector.tensor_scalar(
                        smin, y, 1.0, QSCALE if out_u8 else Q4S,
                        op0=OP.min, op1=OP.mult)
                else:
                    nc.vector.tensor_scalar(
                        smin, y, 1.0, None, op0=OP.min)
                if stage is not None and out_u4:
                    # full-tile quantization (spread rows; garbage rows are
                    # zero in dbtw and never DMA'd): a = d*Q4S + e;
                    # q = round(a) via the 2^23 magic (f32-exact, round-
                    # half-even); e' = a - q; pack two steps' q per byte:
                    # (q_e+8)*16 + (q_o+8)
                    qt = sbs.tile([128, 512], F32, tag="qt", name="qt")
                    if stage_j % 2 == 0:
                        qdst = qprev
                    else:
                        qdst = sbs.tile([128, 512], F32, tag="q2",
                                        name="q2")
                    nc.vector.tensor_mul(qt, smin, dbtw)
                    nc.vector.tensor_add(qt, qt, eq)
                    nc.vector.tensor_scalar(
                        qdst, qt, RMAGIC, None, op0=OP.add)
                    nc.vector.tensor_scalar(
                        qdst, qdst, RMAGIC, None, op0=OP.subtract)
                    nc.vector.tensor_sub(eq, qt, qdst)
                    if stage_j % 2 == 1:
                        nc.vector.scalar_tensor_tensor(
                            qdst, qprev, 16.0, qdst,
                            op0=OP.mult, op1=OP.add)
                        nc.vector.tensor_scalar(
                            stage[:, stage_j // 2, :], qdst,
                            136.0, None, op0=OP.add)
                elif stage is not None:
                    # clipped DELTA into the 32-aligned spread stage; host
                    # integrates positions (f64 cumsum). u8: q=d*QS+QOFF,
                    # the QS factor rides the smin op above for free
                    for c in range(CH):
                        cs2 = slice(32 * c, 32 * c + 2)
                        if out_u8:
                            st = stage[cs2, stage_j, :]
                            qf = sbs.tile([128, 512], F32, tag="qt",
                                          name="qt")
                            qt = qf[cs2, :]
                            nc.vector.tensor_mul(qt, smin[cs2, :],
                                                 dbtw[cs2, :])
                            nc.vector.tensor_add(qt, qt, eq[cs2, :])
                            nc.vector.tensor_scalar(
                                st, qt, QOFF, None, op0=OP.add)
                            nc.vector.tensor_scalar(
                                qt, qt, QOFF, None, op0=OP.add)
                            nc.vector.tensor_sub(eq[cs2, :], qt, st)
                        else:
                            nc.vector.tensor_mul(
                                stage[cs2, stage_j, :],
                                smin[cs2, :], dbtw[cs2, :])
                else:
                    dct = sbs.tile([128, 512], F32, tag="dct", name="dct")
                    nc.vector.tensor_mul(dct, smin, dbtw)
                    nc.vector.tensor_add(pos, pos, dct)
                    posh = sbs.tile([128, 512], F16, tag="posh", name="posh")
                    nc.scalar.activation(posh, pos, AF.Copy)
                    if not no_out:
                        ti = t_idx if not static_out else (
                            t_idx if isinstance(t_idx, int) else 0)
                        for c in range(CH):
                            nc.sync.dma_start(
                                out=out_d[ti, 2 * c:2 * c + 2, :],
                                in_=posh[32 * c:32 * c + 2, :])

            if unroll <= 0:
                for t in range(T):
                    step(t)
            else:
                assert T % unroll == 0
                n_iter = T // unroll
                stp = ctx.enter_context(tc.tile_pool(name="stp", bufs=2))
                with tc.For_i(0, n_iter, 1) as it:
                    stage = stp.tile([128, ucols, 512], ODT,
                                     tag="stage", name="stage")
                    for j in range(unroll):
                        step(None, stage=stage, stage_j=j)
                    if not no_out:
                        ti = it if not static_out else 0
                        for c in range(CH):
                            nc.sync.dma_start(
                                out=out_d[ti, 2 * c:2 * c + 2],
                                in_=stage[32 * c:32 * c + 2, :, :])

    nc.finalize()
    return nc


# ---------------- host side ----------------

_module_cache: dict = {}


def _get_module(T: int, nloc: int, unroll: int, mode: str = "auto"):
    if mode == "auto":
        mode = "u4" if unroll % 2 == 0 else "u8"
    key = (T, nloc, unroll, mode)
    if key not in _module_cache:
        _module_cache[key] = build_module(
            T, nloc, unroll, out_u8=(mode == "u8"), out_u4=(mode == "u4"))
    return _module_cache[key]


def _host_prep(inputs, nloc):
    """Build per-core in_maps from full inputs."""
    N = inputs["init_h"].shape[0]
    n_sh = N // N_CORES
    CH = nloc // 512
    W_ih = np.asarray(inputs["W_ih"], np.float32)
    W_hh = np.asarray(inputs["W_hh"], np.float32)
    b_ih = np.asarray(inputs["b_ih"], np.float32)
    b_hh = np.asarray(inputs["b_hh"], np.float32)
    W_out = np.asarray(inputs["W_out"], np.float32)
    b_out = np.asarray(inputs["b_out"], np.float32)

    import ml_dtypes
    bf16 = ml_dtypes.bfloat16
    ws = WSCALE if USE_FP8 else 1.0
    hdt = ml_dtypes.float8_e4m3 if USE_FP8 else bf16
    wh = np.ascontiguousarray((W_hh.T * ws).reshape(2, 128, 768)).astype(hdt)
    wo = np.ascontiguousarray((W_out.T * ws).reshape(2, 128, 2)).astype(hdt)

    # K=8 input tails: rows 0-1 = delta cols of W_ih, rows 2-6 = ctx cols,
    # row 7 = bias against the ones-row of x (rz tiles: b_ih+b_hh; n tiles:
    # b_ih). Slots 6-7 are bias-only tails adding b_hhn into the P3 PSUM.
    wt = np.zeros((8, 8, 128), np.float32)
    for mt in range(6):
        if mt < 4:
            rows = slice(mt * 128, (mt + 1) * 128)
        else:
            rows = slice(512 + (mt - 4) * 128, 512 + (mt - 3) * 128)
        wt[0:7, mt, :] = W_ih[rows, :].T * ws
        if mt < 4:
            wt[7, mt, :] = ws * (b_ih + b_hh)[rows]
        else:
            wt[7, mt, :] = ws * b_ih[rows]
    for i in range(2):
        rows = slice(512 + i * 128, 512 + (i + 1) * 128)
        wt[7, 6 + i, :] = ws * b_hh[rows]
    wt = wt.astype(bf16)

    # biases: cols 0-3 = (b_ih+b_hh) rz tiles (true scale, added after the
    # ACT 1/WSCALE), 4-5 = b_ih n, 6-7 = b_hh n (pre-scaled: they ride the
    # scaled PSUM values and get divided back at the tanh eviction)
    bv = np.zeros((128, 8), np.float32)
    for mt in range(4):
        bv[:, mt] = (b_ih + b_hh)[mt * 128:(mt + 1) * 128]
    for i in range(2):
        bv[:, 4 + i] = ws * b_ih[512 + i * 128:512 + (i + 1) * 128]
        bv[:, 6 + i] = ws * b_hh[512 + i * 128:512 + (i + 1) * 128]

    wd2 = np.zeros((128, 128), np.float32)
    for c in range(CH):
        for i in range(2):
            for j in range(2):
                wd2[32 * c + i, 32 * c + j] = -0.5 / (MS * MS)

    bpk = np.asarray(b_out, np.float32).reshape(2, 1)

    init_h = np.asarray(inputs["init_h"], np.float32)
    ctx_in = np.asarray(inputs["ctx"], np.float32)
    x0 = np.asarray(inputs["x0"], np.float32)
    y0 = np.asarray(inputs["y0"], np.float32)

    in_maps = []
    for core in range(N_CORES):
        sl = slice(core * n_sh, (core + 1) * n_sh)
        h0 = np.ascontiguousarray(init_h[sl].T.reshape(2, 128, nloc))
        h0b = h0.astype(hdt)
        x0i = np.zeros((8, nloc), bf16)
        x0i[2:7] = ctx_in[sl].T.astype(bf16)
        x0i[7] = bf16(1.0)  # ones-row: carries the rz bias via wt row 7
        pos0 = np.zeros((128, 512), np.float32)
        for c in range(CH):
            pos0[32 * c + 0] = x0[sl].reshape(CH, 512)[c]
            pos0[32 * c + 1] = y0[sl].reshape(CH, 512)[c]
        in_maps.append({
            "h0": h0, "h0b": h0b, "x0i": x0i, "pos0": pos0, "wh": wh,
            "wt": wt, "wo": wo, "wd2": wd2, "bv": bv, "bpk": bpk,
        })
    return in_maps


def _host_unpack(results, T, nloc, unroll=0, x0=None, y0=None):
    CH = nloc // 512
    outs = []
    for ci, r in enumerate(results):
        raw = r["out"]
        if raw.dtype == np.uint8 and unroll > 0 and \
                raw.shape[2] == unroll // 2:
            # packed 4-bit: hi nibble = even step, lo = odd
            hi = (raw >> 4).astype(np.float32) - 8.0
            lo = (raw & 15).astype(np.float32) - 8.0
            arr = np.empty((raw.shape[0], raw.shape[1], unroll, 512),
                           np.float32)
            arr[:, :, 0::2, :] = hi / Q4S
            arr[:, :, 1::2, :] = lo / Q4S
        elif raw.dtype == np.uint8:
            arr = (raw.astype(np.float32) - QOFF) / QSCALE
        else:
            arr = np.asarray(raw, np.float32)
        if unroll > 0:
            # deltas [n_iter, 2CH, unroll, 512], rows 2c+coord; integrate
            a = arr.reshape(T // unroll, CH, 2, unroll, 512)
            a = a.transpose(1, 4, 0, 3, 2)  # ch, s, n_iter, unroll, 2
            d = a.reshape(CH, 512, T, 2).reshape(nloc, T, 2)
            p = np.cumsum(d.astype(np.float64), axis=1)
            sl = slice(ci * nloc, (ci + 1) * nloc)
            p += np.stack([x0[sl], y0[sl]], axis=-1).astype(np.float64)[:, None, :]
            outs.append(p.astype(np.float32))
        else:
            a = arr.reshape(T, CH, 2, 512).transpose(1, 3, 0, 2)
            outs.append(a.reshape(nloc, T, 2))
    return np.concatenate(outs, axis=0)


def _pick_unroll(T: int) -> int:
    for u in (8, 4, 3, 2):
        if T % u == 0:
            return u
    return 1


def kernel(**inputs) -> np.ndarray:
    T = int(inputs["T"])
    N = inputs["init_h"].shape[0]
    nloc = N // N_CORES
    unroll = _pick_unroll(T)
    nc = _get_module(T, nloc, unroll)
    in_maps = _host_prep(inputs, nloc)
    res = run_bass_kernel_spmd(nc, in_maps, core_ids=list(range(N_CORES)))
    return _host_unpack(res.results, T, nloc, unroll,
                        x0=np.asarray(inputs["x0"], np.float32),
                        y0=np.asarray(inputs["y0"], np.float32))

